# revision 3
# baseline (speedup 1.0000x reference)
"""GQA attention (B=2, T=2048, H=2048, 16 heads / 4 kv heads, RoPE, causal)
distributed over 8 trn2 NeuronCores.

Sharding: 2 q-heads + their kv head per core (tensor parallel). Each core
computes q/k/v projections for its heads, RoPE, causal attention, then an
AllToAll reshards attention output from head-parallel to token-parallel so
the o-projection runs against the full wo with no AllReduce; each core
returns its own 512 tokens of the final output and the host concatenates.

All matmuls run in float32r (full-rate fp32 on the PE array, ~1e-3 rel).
Layout trick: scores are computed transposed (sT[k,q]) so softmax+PV need
no on-chip transposes; row sums come from a ones-vector matmul; exp runs
without max-subtraction (scores are O(1) here, fp32 exp cannot overflow).
"""

import sys

for _p in ("/opt/trn_rl_repo", "/root/.axon_site/_ro/trn_rl_repo"):
    if _p not in sys.path:
        sys.path.append(_p)

import numpy as np

import concourse.bacc as bacc
import concourse.mybir as mybir
import concourse.tile as tile
from concourse.bass_utils import run_bass_kernel_spmd

B, T, H = 2, 2048, 2048
N_HEAD, N_KV_HEAD = 16, 4
HD = H // N_HEAD  # 128
TOK = B * T  # 4096
CORES = 8
HPC = N_HEAD // CORES  # 2 q heads per core
DPC = HPC * HD  # 256 q dims per core
OWN = T // CORES  # 256 own tokens per batch per core
SCALE = HD**-0.5
ROPE_THETA = 10000.0

F32 = mybir.dt.float32
F32R = mybir.dt.float32r
EXP = mybir.ActivationFunctionType.Exp

_CACHE = {}


def _build_nc():
    nc = bacc.Bacc("TRN2", target_bir_lowering=False, debug=False, num_devices=CORES)

    x_e = nc.declare_dram_parameter("x", [TOK, H], F32, isOutput=False)
    wq_e = nc.declare_dram_parameter("wq", [H, DPC], F32, isOutput=False)
    wk_e = nc.declare_dram_parameter("wk", [H, HD], F32, isOutput=False)
    wv_e = nc.declare_dram_parameter("wv", [H, HD], F32, isOutput=False)
    wo_e = nc.declare_dram_parameter("wo", [H, H], F32, isOutput=False)
    cos_e = nc.declare_dram_parameter("cosT", [HD, T], F32, isOutput=False)
    sin_e = nc.declare_dram_parameter("sinT", [HD, T], F32, isOutput=False)
    msk_e = nc.declare_dram_parameter("dmask", [128, 4 * 512], F32, isOutput=False)
    id_e = nc.declare_dram_parameter("ident", [128, 128], F32, isOutput=False)
    out_e = nc.declare_dram_parameter("out", [B, OWN, H], F32, isOutput=True)

    with tile.TileContext(nc) as tc:
        with (
            tc.tile_pool(name="dram", bufs=1, space="DRAM") as dpool,
            tc.tile_pool(name="const", bufs=1) as cpool,
        ):
            send = [
                dpool.tile([CORES, DPC, OWN], F32, name=f"send{b}", tag=f"send{b}")
                for b in range(B)
            ]
            recv = [
                dpool.tile(
                    [CORES, DPC, OWN], F32, name=f"recv{b}", tag=f"recv{b}"
                )
                for b in range(B)
            ]

            ident = cpool.tile([128, 128], F32)
            nc.sync.dma_start(out=ident[:, :], in_=id_e[:, :])
            cosT = cpool.tile([HD, T], F32)
            nc.sync.dma_start(out=cosT[:, :], in_=cos_e[:, :])
            sinT = cpool.tile([HD, T], F32)
            nc.sync.dma_start(out=sinT[:, :], in_=sin_e[:, :])
            dmask = cpool.tile([128, 4 * 512], F32)
            nc.sync.dma_start(out=dmask[:, :], in_=msk_e[:, :])

            ones_f32 = cpool.tile([128, 1], F32)
            nc.vector.memset(ones_f32[:, :], 1.0)
            ones_col = cpool.tile([128, 1], F32R)
            nc.vector.tensor_copy(ones_col[:, :], ones_f32[:, :])
            onesr_f32 = cpool.tile([1, 128], F32)
            nc.vector.memset(onesr_f32[:, :], 1.0)
            ones_row = cpool.tile([1, 128], F32R)
            nc.vector.tensor_copy(ones_row[:, :], onesr_f32[:, :])

            # weights -> f32r (DVE rounds on write)
            wq_r = cpool.tile([128, 16, DPC], F32R)
            wk_r = cpool.tile([128, 16, HD], F32R)
            wv_r = cpool.tile([128, 16, HD], F32R)
            for k in range(16):
                stg = cpool.tile([128, DPC], F32, tag="wstg", bufs=2)
                nc.sync.dma_start(out=stg[:, :], in_=wq_e[k * 128 : (k + 1) * 128, :])
                nc.vector.tensor_copy(wq_r[:, k, :], stg[:, :])
                stg2 = cpool.tile([128, HD], F32, tag="wstg2", bufs=2)
                nc.sync.dma_start(out=stg2[:, :], in_=wk_e[k * 128 : (k + 1) * 128, :])
                nc.vector.tensor_copy(wk_r[:, k, :], stg2[:, :])
                stg3 = cpool.tile([128, HD], F32, tag="wstg3", bufs=2)
                nc.sync.dma_start(out=stg3[:, :], in_=wv_e[k * 128 : (k + 1) * 128, :])
                nc.vector.tensor_copy(wv_r[:, k, :], stg3[:, :])

            for b in range(B):
                with tc.tile_pool(name=f"pb{b}", bufs=1) as pp:
                    qTr = pp.tile([128, HPC, T], F32R)
                    kTr = pp.tile([128, T], F32R)
                    vtm = pp.tile([128, 16, HD], F32R)  # token-major v tiles

                    with tc.tile_pool(name=f"pp{b}", bufs=2, space="PSUM") as ppsum:
                        for ch in range(4):  # 512-token chunks of this batch
                            row0 = b * T + ch * 512
                            xT = pp.tile(
                                [128, 16, 512], F32R, tag="xT", bufs=1, name="xT"
                            )
                            for tt in range(4):
                                for fb in range(4):
                                    xin = pp.tile(
                                        [128, 512], F32, tag="xin", bufs=6, name="xin"
                                    )
                                    nc.sync.dma_start(
                                        out=xin[:, :],
                                        in_=x_e[
                                            row0 + tt * 128 : row0 + (tt + 1) * 128,
                                            fb * 512 : (fb + 1) * 512,
                                        ],
                                    )
                                    for j in range(4):
                                        ft = fb * 4 + j
                                        pst = ppsum.tile(
                                            [128, 128], F32, tag="tr", name="pst"
                                        )
                                        nc.tensor.transpose(
                                            pst[:, :],
                                            xin[:, j * 128 : (j + 1) * 128],
                                            ident[:, :],
                                        )
                                        nc.vector.tensor_copy(
                                            xT[:, ft, tt * 128 : (tt + 1) * 128],
                                            pst[:, :],
                                        )

                            tsl = slice(ch * 512, (ch + 1) * 512)

                            def rope(dst, ps):
                                sb = pp.tile([128, 512], F32, tag="rsb", bufs=2, name="rsb")
                                nc.vector.tensor_copy(sb[:, :], ps[:, :])
                                rot = pp.tile([128, 512], F32, tag="rrot", bufs=2, name="rrot")
                                nc.sync.dma_start(out=rot[0:64, :], in_=sb[64:128, :])
                                nc.sync.dma_start(out=rot[64:128, :], in_=sb[0:64, :])
                                t1 = pp.tile([128, 512], F32, tag="rt1", bufs=2, name="rt1")
                                nc.vector.tensor_mul(t1[:, :], sb[:, :], cosT[:, tsl])
                                t2 = pp.tile([128, 512], F32, tag="rt2", bufs=2, name="rt2")
                                nc.vector.tensor_mul(t2[:, :], rot[:, :], sinT[:, tsl])
                                nc.vector.tensor_add(dst, t1[:, :], t2[:, :])

                            for m in range(HPC):
                                psq = ppsum.tile([128, 512], F32, tag="pj", name="psq")
                                for k in range(16):
                                    nc.tensor.matmul(
                                        psq[:, :],
                                        wq_r[:, k, m * 128 : (m + 1) * 128],
                                        xT[:, k, :],
                                        start=(k == 0),
                                        stop=(k == 15),
                                    )
                                rope(qTr[:, m, tsl], psq)

                            psk = ppsum.tile([128, 512], F32, tag="pj", name="psk")
                            for k in range(16):
                                nc.tensor.matmul(
                                    psk[:, :],
                                    wk_r[:, k, :],
                                    xT[:, k, :],
                                    start=(k == 0),
                                    stop=(k == 15),
                                )
                            rope(kTr[:, tsl], psk)

                            psv = ppsum.tile([128, 512], F32, tag="pj", name="psv")
                            for k in range(16):
                                nc.tensor.matmul(
                                    psv[:, :],
                                    wv_r[:, k, :],
                                    xT[:, k, :],
                                    start=(k == 0),
                                    stop=(k == 15),
                                )
                            vsb = pp.tile([128, 512], F32, tag="vsb", bufs=2, name="vsb")
                            nc.vector.tensor_copy(vsb[:, :], psv[:, :])
                            for s in range(4):
                                pst = ppsum.tile([128, 128], F32, tag="tr", name="pstv")
                                nc.tensor.transpose(
                                    pst[:, :], vsb[:, s * 128 : (s + 1) * 128], ident[:, :]
                                )
                                nc.vector.tensor_copy(vtm[:, ch * 4 + s, :], pst[:, :])

                    # ---- attention for this batch ----
                    with (
                        tc.tile_pool(name=f"ap{b}", bufs=2, space="PSUM") as apsum,
                        tc.tile_pool(name=f"ab{b}", bufs=1) as ap,
                    ):
                        for h in range(HPC):
                            for qb in range(4):
                                qsl = slice(qb * 512, (qb + 1) * 512)
                                nkt = 4 * (qb + 1)
                                pso = apsum.tile([128, 512], F32, tag="o", name="pso")
                                psl = apsum.tile([1, 512], F32, tag="l", name="psl")
                                for kt in range(nkt):
                                    pss = apsum.tile([128, 512], F32, tag="s", name="pss")
                                    nc.tensor.matmul(
                                        pss[:, :],
                                        kTr[:, kt * 128 : (kt + 1) * 128],
                                        qTr[:, h, qsl],
                                        start=True,
                                        stop=True,
                                    )
                                    r = kt - 4 * qb
                                    pT = ap.tile([128, 512], F32R, tag="pT", bufs=3, name="pT")
                                    if r >= 0:
                                        et = ap.tile(
                                            [128, 512], F32, tag="et", bufs=2, name="et"
                                        )
                                        nc.scalar.activation(et[:, :], pss[:, :], EXP)
                                        nc.vector.tensor_mul(
                                            pT[:, :],
                                            et[:, :],
                                            dmask[:, r * 512 : (r + 1) * 512],
                                        )
                                    else:
                                        nc.scalar.activation(pT[:, :], pss[:, :], EXP)
                                    nc.tensor.matmul(
                                        pso[:, :],
                                        vtm[:, kt, :],
                                        pT[:, :],
                                        start=(kt == 0),
                                        stop=(kt == nkt - 1),
                                    )
                                    nc.tensor.matmul(
                                        psl[0:1, :],
                                        ones_col[:, :],
                                        pT[:, :],
                                        start=(kt == 0),
                                        stop=(kt == nkt - 1),
                                    )
                                linv = ap.tile([1, 512], F32, tag="li", bufs=2, name="linv")
                                nc.vector.reciprocal(linv[:, :], psl[0:1, :])
                                linv_r = ap.tile([1, 512], F32R, tag="lir", bufs=2, name="linvr")
                                nc.vector.tensor_copy(linv_r[:, :], linv[:, :])
                                psb = apsum.tile([128, 512], F32, tag="bc", bufs=1, name="psb")
                                nc.tensor.matmul(
                                    psb[:, :],
                                    ones_row[:, :],
                                    linv_r[:, :],
                                    start=True,
                                    stop=True,
                                )
                                lbc = ap.tile([128, 512], F32, tag="lbc", bufs=2, name="lbc")
                                nc.vector.tensor_copy(lbc[:, :], psb[:, :])
                                aout = ap.tile([128, 512], F32, tag="ao", bufs=2, name="aout")
                                nc.vector.tensor_mul(aout[:, :], pso[:, :], lbc[:, :])
                                for half in range(2):
                                    j = 2 * qb + half
                                    nc.sync.dma_start(
                                        out=send[b][j, h * 128 : (h + 1) * 128, :],
                                        in_=aout[:, half * 256 : (half + 1) * 256],
                                    )

                nc.gpsimd.collective_compute(
                    "AllToAll",
                    mybir.AluOpType.bypass,
                    replica_groups=[list(range(CORES))],
                    ins=[send[b].opt()],
                    outs=[recv[b].opt()],
                )

            # ---- o-projection on own tokens ----
            with (
                tc.tile_pool(name="op", bufs=1) as op,
                tc.tile_pool(name="ops", bufs=1, space="PSUM") as opsum,
            ):
                ao_r = op.tile([128, B, 16, OWN], F32R)
                for b in range(B):
                    for k in range(16):
                        stg = op.tile([128, OWN], F32, tag="aostg", bufs=3, name="aostg")
                        nc.sync.dma_start(
                            out=stg[:, :],
                            in_=recv[b][k // 2, (k % 2) * 128 : (k % 2) * 128 + 128, :],
                        )
                        nc.vector.tensor_copy(ao_r[:, b, k, :], stg[:, :])

                for nb in range(4):
                    nsl = slice(nb * 512, (nb + 1) * 512)
                    pss = [
                        opsum.tile([128, 512], F32, tag=f"op{i}", bufs=2, name=f"ops{i}")
                        for i in range(4)
                    ]
                    for k in range(16):
                        wst = op.tile([128, 512], F32, tag="wst", bufs=4, name="wst")
                        nc.sync.dma_start(
                            out=wst[:, :], in_=wo_e[k * 128 : (k + 1) * 128, nsl]
                        )
                        wr = op.tile([128, 512], F32R, tag="wr", bufs=4, name="wr")
                        nc.vector.tensor_copy(wr[:, :], wst[:, :])
                        for b in range(B):
                            for tt in range(2):
                                nc.tensor.matmul(
                                    pss[2 * b + tt][:, :],
                                    ao_r[:, b, k, tt * 128 : (tt + 1) * 128],
                                    wr[:, :],
                                    start=(k == 0),
                                    stop=(k == 15),
                                )
                    for b in range(B):
                        for tt in range(2):
                            osb = op.tile([128, 512], F32, tag="osb", bufs=4, name="osb")
                            nc.vector.tensor_copy(osb[:, :], pss[2 * b + tt][:, :])
                            nc.sync.dma_start(
                                out=out_e[b, tt * 128 : (tt + 1) * 128, nsl],
                                in_=osb[:, :],
                            )

    nc.compile()
    return nc


def _host_tables():
    inv_freq = 1.0 / (ROPE_THETA ** (np.arange(0, HD, 2, dtype=np.float64) / HD))
    pos = np.arange(T, dtype=np.float64)
    freqs = pos[:, None] * inv_freq[None, :]  # [T, 64]
    emb = np.concatenate([freqs, freqs], axis=-1)  # [T, 128]
    cosT = np.cos(emb).T.astype(np.float32)  # [128, T]
    sinT = np.sin(emb).T.astype(np.float32)
    sinT[:64, :] *= -1.0  # sign of the rotate-half fold
    # diagonal-band causal masks: dmask[r][k', q'] = 1 if q' >= 128 r + k'
    q = np.arange(512)[None, :]
    kk = np.arange(128)[:, None]
    dm = np.concatenate(
        [(q >= 128 * r + kk).astype(np.float32) for r in range(4)], axis=1
    )  # [128, 2048]
    ident = np.eye(128, dtype=np.float32)
    return cosT, sinT, dm, ident


def _run(inputs, trace=False):
    if "nc" not in _CACHE:
        _CACHE["nc"] = _build_nc()
    nc = _CACHE["nc"]

    x = np.ascontiguousarray(inputs["x"], dtype=np.float32).reshape(TOK, H)
    wq = np.asarray(inputs["wq"], dtype=np.float32) * np.float32(SCALE)
    wk = np.asarray(inputs["wk"], dtype=np.float32)
    wv = np.asarray(inputs["wv"], dtype=np.float32)
    wo = np.ascontiguousarray(inputs["wo"], dtype=np.float32)
    cosT, sinT, dm, ident = _host_tables()

    in_maps = []
    for c in range(CORES):
        kv = c // 2
        in_maps.append(
            {
                "x": x,
                "wq": np.ascontiguousarray(wq[:, c * DPC : (c + 1) * DPC]),
                "wk": np.ascontiguousarray(wk[:, kv * HD : (kv + 1) * HD]),
                "wv": np.ascontiguousarray(wv[:, kv * HD : (kv + 1) * HD]),
                "wo": wo,
                "cosT": cosT,
                "sinT": sinT,
                "dmask": dm,
                "ident": ident,
            }
        )

    res = run_bass_kernel_spmd(
        nc, in_maps, core_ids=list(range(CORES)), trace=trace
    )
    out = np.empty((B, T, H), dtype=np.float32)
    for c in range(CORES):
        o = res.results[c]["out"]  # [B, OWN, H]
        for b in range(B):
            out[b, c * OWN : (c + 1) * OWN, :] = o[b]
    return out, res


def kernel(**inputs) -> np.ndarray:
    out, _ = _run(inputs, trace=False)
    return out


# revision 4
# speedup vs baseline: 1.2556x; 1.2556x over previous
"""GQA attention (B=2, T=2048, H=2048, 16 heads / 4 kv heads, RoPE, causal)
distributed over 8 trn2 NeuronCores.

Sharding: 2 q-heads + their kv head per core (tensor parallel). Each core
computes q/k/v projections for its heads, RoPE, causal attention, then an
AllToAll reshards attention output from head-parallel to token-parallel so
the o-projection runs against the full wo with no AllReduce; each core
returns its own 512 tokens of the final output and the host concatenates.

All matmuls run in float32r (full-rate fp32 on the PE array, ~1e-3 rel).
Layout trick: scores are computed transposed (sT[k,q]) so softmax+PV need
no on-chip transposes; row sums come from a ones-vector matmul; exp runs
without max-subtraction (scores are O(1) here, fp32 exp cannot overflow).
The attention inner loop is software-pipelined (scores issue 2 k-tiles
ahead of PV) so the PE never waits on ScalarE's exp.
"""

import sys

for _p in ("/opt/trn_rl_repo", "/root/.axon_site/_ro/trn_rl_repo"):
    if _p not in sys.path:
        sys.path.append(_p)

import numpy as np

import concourse.bacc as bacc
import concourse.mybir as mybir
import concourse.tile as tile
from concourse.bass_utils import run_bass_kernel_spmd

B, T, H = 2, 2048, 2048
N_HEAD, N_KV_HEAD = 16, 4
HD = H // N_HEAD  # 128
TOK = B * T  # 4096
CORES = 8
HPC = N_HEAD // CORES  # 2 q heads per core
DPC = HPC * HD  # 256 q dims per core
OWN = T // CORES  # 256 own tokens per batch per core
SCALE = HD**-0.5
ROPE_THETA = 10000.0

F32 = mybir.dt.float32
F32R = mybir.dt.float32r
EXP = mybir.ActivationFunctionType.Exp

_CACHE = {}


def _build_nc():
    nc = bacc.Bacc("TRN2", target_bir_lowering=False, debug=False, num_devices=CORES)

    x_e = nc.declare_dram_parameter("x", [TOK, H], F32, isOutput=False)
    wq_e = nc.declare_dram_parameter("wq", [H, DPC], F32, isOutput=False)
    wk_e = nc.declare_dram_parameter("wk", [H, HD], F32, isOutput=False)
    wv_e = nc.declare_dram_parameter("wv", [H, HD], F32, isOutput=False)
    wo_e = nc.declare_dram_parameter("wo", [H, H], F32, isOutput=False)
    cos_e = nc.declare_dram_parameter("cosT", [HD, T], F32, isOutput=False)
    sin_e = nc.declare_dram_parameter("sinT", [HD, T], F32, isOutput=False)
    msk_e = nc.declare_dram_parameter("dmask", [128, 4 * 512], F32, isOutput=False)
    id_e = nc.declare_dram_parameter("ident", [128, 128], F32, isOutput=False)
    out_e = nc.declare_dram_parameter("out", [B, OWN, H], F32, isOutput=True)

    with tile.TileContext(nc) as tc:
        with (
            tc.tile_pool(name="dram", bufs=1, space="DRAM") as dpool,
            tc.tile_pool(name="const", bufs=1) as cpool,
        ):
            send = [
                dpool.tile([CORES, DPC, OWN], F32, name=f"send{b}", tag=f"send{b}")
                for b in range(B)
            ]
            recv = [
                dpool.tile([CORES, DPC, OWN], F32, name=f"recv{b}", tag=f"recv{b}")
                for b in range(B)
            ]

            ident = cpool.tile([128, 128], F32)
            nc.sync.dma_start(out=ident[:, :], in_=id_e[:, :])
            cosT = cpool.tile([HD, T], F32)
            nc.sync.dma_start(out=cosT[:, :], in_=cos_e[:, :])
            sinT = cpool.tile([HD, T], F32)
            nc.sync.dma_start(out=sinT[:, :], in_=sin_e[:, :])
            dmask = cpool.tile([128, 4 * 512], F32)
            nc.sync.dma_start(out=dmask[:, :], in_=msk_e[:, :])

            ones_f32 = cpool.tile([128, 1], F32)
            nc.vector.memset(ones_f32[:, :], 1.0)
            ones_col = cpool.tile([128, 1], F32R)
            nc.vector.tensor_copy(ones_col[:, :], ones_f32[:, :])

            # weights -> f32r (gpsimd rounds on write; DVE stays free)
            wq_r = cpool.tile([128, 16, DPC], F32R)
            wk_r = cpool.tile([128, 16, HD], F32R)
            wv_r = cpool.tile([128, 16, HD], F32R)
            for k in range(16):
                stg = cpool.tile([128, DPC], F32, tag="wstg", bufs=2, name="wstg")
                nc.sync.dma_start(out=stg[:, :], in_=wq_e[k * 128 : (k + 1) * 128, :])
                nc.gpsimd.tensor_copy(wq_r[:, k, :], stg[:, :])
                stg2 = cpool.tile([128, HD], F32, tag="wstg2", bufs=2, name="wstg2")
                nc.sync.dma_start(out=stg2[:, :], in_=wk_e[k * 128 : (k + 1) * 128, :])
                nc.gpsimd.tensor_copy(wk_r[:, k, :], stg2[:, :])
                stg3 = cpool.tile([128, HD], F32, tag="wstg3", bufs=2, name="wstg3")
                nc.sync.dma_start(out=stg3[:, :], in_=wv_e[k * 128 : (k + 1) * 128, :])
                nc.gpsimd.tensor_copy(wv_r[:, k, :], stg3[:, :])

            for b in range(B):
                with tc.tile_pool(name=f"pb{b}", bufs=1) as pp:
                    qTr = pp.tile([128, HPC, T], F32R)
                    kTr = pp.tile([128, T], F32R)
                    vtm = pp.tile([128, 16, HD], F32R)  # token-major v tiles

                    with tc.tile_pool(name=f"pp{b}", bufs=2, space="PSUM") as ppsum:
                        for ch in range(4):  # 512-token chunks of this batch
                            row0 = b * T + ch * 512
                            # xT chunk, layout [tt, ft, 128tok]
                            xT = pp.tile(
                                [128, 4, 16, 128], F32R, tag="xT", bufs=1, name="xT"
                            )
                            for tt in range(4):
                                for fb in range(4):
                                    xin = pp.tile(
                                        [128, 512], F32, tag="xin", bufs=6, name="xin"
                                    )
                                    nc.sync.dma_start(
                                        out=xin[:, :],
                                        in_=x_e[
                                            row0 + tt * 128 : row0 + (tt + 1) * 128,
                                            fb * 512 : (fb + 1) * 512,
                                        ],
                                    )
                                    bank = ppsum.tile(
                                        [128, 512], F32, tag="tr", name="trbank"
                                    )
                                    for j in range(4):
                                        nc.tensor.transpose(
                                            bank[:, j * 128 : (j + 1) * 128],
                                            xin[:, j * 128 : (j + 1) * 128],
                                            ident[:, :],
                                        )
                                    nc.vector.tensor_copy(
                                        xT[:, tt, fb * 4 : (fb + 1) * 4, :], bank[:, :]
                                    )

                            tsl = slice(ch * 512, (ch + 1) * 512)

                            def rope(dst, ps):
                                sb = pp.tile([128, 512], F32, tag="rsb", bufs=2, name="rsb")
                                nc.vector.tensor_copy(sb[:, :], ps[:, :])
                                rot = pp.tile([128, 512], F32, tag="rrot", bufs=2, name="rrot")
                                nc.sync.dma_start(out=rot[0:64, :], in_=sb[64:128, :])
                                nc.sync.dma_start(out=rot[64:128, :], in_=sb[0:64, :])
                                t1 = pp.tile([128, 512], F32, tag="rt1", bufs=2, name="rt1")
                                nc.vector.tensor_mul(t1[:, :], sb[:, :], cosT[:, tsl])
                                t2 = pp.tile([128, 512], F32, tag="rt2", bufs=2, name="rt2")
                                nc.gpsimd.tensor_mul(t2[:, :], rot[:, :], sinT[:, tsl])
                                nc.vector.tensor_add(dst, t1[:, :], t2[:, :])

                            for m in range(HPC):
                                psq = ppsum.tile([128, 512], F32, tag="pj", name="psq")
                                for k in range(16):
                                    nc.tensor.matmul(
                                        psq[:, :],
                                        wq_r[:, k, m * 128 : (m + 1) * 128],
                                        xT[:, :, k, :],
                                        start=(k == 0),
                                        stop=(k == 15),
                                    )
                                rope(qTr[:, m, tsl], psq)

                            psk = ppsum.tile([128, 512], F32, tag="pj", name="psk")
                            for k in range(16):
                                nc.tensor.matmul(
                                    psk[:, :],
                                    wk_r[:, k, :],
                                    xT[:, :, k, :],
                                    start=(k == 0),
                                    stop=(k == 15),
                                )
                            rope(kTr[:, tsl], psk)

                            psv = ppsum.tile([128, 512], F32, tag="pj", name="psv")
                            for k in range(16):
                                nc.tensor.matmul(
                                    psv[:, :],
                                    wv_r[:, k, :],
                                    xT[:, :, k, :],
                                    start=(k == 0),
                                    stop=(k == 15),
                                )
                            vsb = pp.tile([128, 512], F32, tag="vsb", bufs=2, name="vsb")
                            nc.vector.tensor_copy(vsb[:, :], psv[:, :])
                            vbank = ppsum.tile([128, 512], F32, tag="tr", name="vbank")
                            for s in range(4):
                                nc.tensor.transpose(
                                    vbank[:, s * 128 : (s + 1) * 128],
                                    vsb[:, s * 128 : (s + 1) * 128],
                                    ident[:, :],
                                )
                            nc.vector.tensor_copy(
                                vtm[:, ch * 4 : (ch + 1) * 4, :], vbank[:, :]
                            )

                    # ---- attention for this batch (sw-pipelined, lookahead 2) ----
                    with (
                        tc.tile_pool(name=f"ap{b}", bufs=2, space="PSUM") as apsum,
                        tc.tile_pool(name=f"ab{b}", bufs=1) as ap,
                    ):
                        for h in range(HPC):
                            for qb in range(4):
                                qsl = slice(qb * 512, (qb + 1) * 512)
                                nkt = 4 * (qb + 1)
                                pso = apsum.tile([128, 512], F32, tag="o", name="pso")
                                psl = apsum.tile([1, 512], F32, tag="l", name="psl")
                                pts = {}

                                def score(kt):
                                    pss = apsum.tile(
                                        [128, 512], F32, tag="s", bufs=4, name="pss"
                                    )
                                    nc.tensor.matmul(
                                        pss[:, :],
                                        kTr[:, kt * 128 : (kt + 1) * 128],
                                        qTr[:, h, qsl],
                                        start=True,
                                        stop=True,
                                    )
                                    r = kt - 4 * qb
                                    pT = ap.tile(
                                        [128, 512], F32R, tag="pT", bufs=6, name="pT"
                                    )
                                    if r >= 0:
                                        et = ap.tile(
                                            [128, 512], F32, tag="et", bufs=3, name="et"
                                        )
                                        nc.scalar.activation(et[:, :], pss[:, :], EXP)
                                        nc.vector.tensor_mul(
                                            pT[:, :],
                                            et[:, :],
                                            dmask[:, r * 512 : (r + 1) * 512],
                                        )
                                    else:
                                        nc.scalar.activation(pT[:, :], pss[:, :], EXP)
                                    pts[kt] = pT

                                def pv(kt):
                                    pT = pts.pop(kt)
                                    nc.tensor.matmul(
                                        pso[:, :],
                                        vtm[:, kt, :],
                                        pT[:, :],
                                        start=(kt == 0),
                                        stop=(kt == nkt - 1),
                                    )
                                    nc.tensor.matmul(
                                        psl[0:1, :],
                                        ones_col[:, :],
                                        pT[:, :],
                                        start=(kt == 0),
                                        stop=(kt == nkt - 1),
                                    )

                                LA = 2  # scores run this many k-tiles ahead of PV
                                for kt in range(nkt):
                                    score(kt)
                                    if kt >= LA:
                                        pv(kt - LA)
                                for kt in range(max(0, nkt - LA), nkt):
                                    pv(kt)

                                linv = ap.tile([1, 512], F32, tag="li", bufs=2, name="linv")
                                nc.vector.reciprocal(linv[:, :], psl[0:1, :])
                                lbc = ap.tile([128, 512], F32, tag="lbc", bufs=2, name="lbc")
                                nc.gpsimd.partition_broadcast(lbc[:, :], linv[0:1, :])
                                aout = ap.tile([128, 512], F32, tag="ao", bufs=2, name="aout")
                                nc.vector.tensor_mul(aout[:, :], pso[:, :], lbc[:, :])
                                for half in range(2):
                                    j = 2 * qb + half
                                    nc.sync.dma_start(
                                        out=send[b][j, h * 128 : (h + 1) * 128, :],
                                        in_=aout[:, half * 256 : (half + 1) * 256],
                                    )

                nc.gpsimd.collective_compute(
                    "AllToAll",
                    mybir.AluOpType.bypass,
                    replica_groups=[list(range(CORES))],
                    ins=[send[b].opt()],
                    outs=[recv[b].opt()],
                )

            # ---- o-projection on own tokens ----
            with (
                tc.tile_pool(name="op", bufs=1) as op,
                tc.tile_pool(name="ops", bufs=1, space="PSUM") as opsum,
            ):
                ao_r = op.tile([128, B, 16, OWN], F32R)
                for b in range(B):
                    for k in range(16):
                        stg = op.tile([128, OWN], F32, tag="aostg", bufs=3, name="aostg")
                        nc.sync.dma_start(
                            out=stg[:, :],
                            in_=recv[b][k // 2, (k % 2) * 128 : (k % 2) * 128 + 128, :],
                        )
                        nc.gpsimd.tensor_copy(ao_r[:, b, k, :], stg[:, :])

                for nb in range(4):
                    nsl = slice(nb * 512, (nb + 1) * 512)
                    # all 16 wo k-tiles of this n-block stay resident so the
                    # two batches reuse them (A2A of b=1 hides behind b=0 MMs)
                    wrs = []
                    for k in range(16):
                        wst = op.tile([128, 512], F32, tag="wst", bufs=4, name="wst")
                        nc.sync.dma_start(
                            out=wst[:, :], in_=wo_e[k * 128 : (k + 1) * 128, nsl]
                        )
                        wr = op.tile(
                            [128, 512], F32R, tag=f"wr{k}", bufs=2, name=f"wr{k}"
                        )
                        nc.gpsimd.tensor_copy(wr[:, :], wst[:, :])
                        wrs.append(wr)
                    for b in range(B):
                        for tt in range(2):
                            ps = opsum.tile(
                                [128, 512],
                                F32,
                                tag=f"op{2 * b + tt}",
                                bufs=2,
                                name=f"ops{2 * b + tt}",
                            )
                            for k in range(16):
                                nc.tensor.matmul(
                                    ps[:, :],
                                    ao_r[:, b, k, tt * 128 : (tt + 1) * 128],
                                    wrs[k][:, :],
                                    start=(k == 0),
                                    stop=(k == 15),
                                )
                            osb = op.tile([128, 512], F32, tag="osb", bufs=4, name="osb")
                            nc.vector.tensor_copy(osb[:, :], ps[:, :])
                            nc.sync.dma_start(
                                out=out_e[b, tt * 128 : (tt + 1) * 128, nsl],
                                in_=osb[:, :],
                            )

    nc.compile()
    return nc


def _host_tables():
    inv_freq = 1.0 / (ROPE_THETA ** (np.arange(0, HD, 2, dtype=np.float64) / HD))
    pos = np.arange(T, dtype=np.float64)
    freqs = pos[:, None] * inv_freq[None, :]  # [T, 64]
    emb = np.concatenate([freqs, freqs], axis=-1)  # [T, 128]
    cosT = np.cos(emb).T.astype(np.float32)  # [128, T]
    sinT = np.sin(emb).T.astype(np.float32)
    sinT[:64, :] *= -1.0  # sign of the rotate-half fold
    # diagonal-band causal masks: dmask[r][k', q'] = 1 if q' >= 128 r + k'
    q = np.arange(512)[None, :]
    kk = np.arange(128)[:, None]
    dm = np.concatenate(
        [(q >= 128 * r + kk).astype(np.float32) for r in range(4)], axis=1
    )  # [128, 2048]
    ident = np.eye(128, dtype=np.float32)
    return cosT, sinT, dm, ident


def _run(inputs, trace=False):
    if "nc" not in _CACHE:
        _CACHE["nc"] = _build_nc()
    nc = _CACHE["nc"]

    x = np.ascontiguousarray(inputs["x"], dtype=np.float32).reshape(TOK, H)
    wq = np.asarray(inputs["wq"], dtype=np.float32) * np.float32(SCALE)
    wk = np.asarray(inputs["wk"], dtype=np.float32)
    wv = np.asarray(inputs["wv"], dtype=np.float32)
    wo = np.ascontiguousarray(inputs["wo"], dtype=np.float32)
    cosT, sinT, dm, ident = _host_tables()

    in_maps = []
    for c in range(CORES):
        kv = c // 2
        in_maps.append(
            {
                "x": x,
                "wq": np.ascontiguousarray(wq[:, c * DPC : (c + 1) * DPC]),
                "wk": np.ascontiguousarray(wk[:, kv * HD : (kv + 1) * HD]),
                "wv": np.ascontiguousarray(wv[:, kv * HD : (kv + 1) * HD]),
                "wo": wo,
                "cosT": cosT,
                "sinT": sinT,
                "dmask": dm,
                "ident": ident,
            }
        )

    res = run_bass_kernel_spmd(nc, in_maps, core_ids=list(range(CORES)), trace=trace)
    out = np.empty((B, T, H), dtype=np.float32)
    for c in range(CORES):
        o = res.results[c]["out"]  # [B, OWN, H]
        for b in range(B):
            out[b, c * OWN : (c + 1) * OWN, :] = o[b]
    return out, res


def kernel(**inputs) -> np.ndarray:
    out, _ = _run(inputs, trace=False)
    return out


# revision 5
# speedup vs baseline: 1.3442x; 1.0705x over previous
"""GQA attention (B=2, T=2048, H=2048, 16 heads / 4 kv heads, RoPE, causal)
distributed over 8 trn2 NeuronCores.

Sharding: 2 q-heads + their kv head per core (tensor parallel). Each core
computes q/k/v projections for its heads, RoPE, causal attention, then an
AllToAll reshards attention output from head-parallel to token-parallel so
the o-projection runs against the full wo with no AllReduce; each core
returns its own 512 tokens of the final output and the host concatenates.

All matmuls run in float32r (full-rate fp32 on the PE array, ~1e-3 rel).
Layout trick: scores are computed transposed (sT[k,q]) so softmax+PV need
no on-chip transposes; row sums come from a ones-vector matmul; exp runs
without max-subtraction (scores are O(1) here, fp32 exp cannot overflow).
The attention inner loop is software-pipelined (scores issue 2 k-tiles
ahead of PV) so the PE never waits on ScalarE's exp.
"""

import sys

for _p in ("/opt/trn_rl_repo", "/root/.axon_site/_ro/trn_rl_repo"):
    if _p not in sys.path:
        sys.path.append(_p)

import numpy as np

import concourse.bacc as bacc
import concourse.mybir as mybir
import concourse.tile as tile
from concourse.bass_utils import run_bass_kernel_spmd

B, T, H = 2, 2048, 2048
N_HEAD, N_KV_HEAD = 16, 4
HD = H // N_HEAD  # 128
TOK = B * T  # 4096
CORES = 8
HPC = N_HEAD // CORES  # 2 q heads per core
DPC = HPC * HD  # 256 q dims per core
OWN = T // CORES  # 256 own tokens per batch per core
SCALE = HD**-0.5
ROPE_THETA = 10000.0

F32 = mybir.dt.float32
F32R = mybir.dt.float32r
EXP = mybir.ActivationFunctionType.Exp

_CACHE = {}


def _build_nc():
    nc = bacc.Bacc("TRN2", target_bir_lowering=False, debug=False, num_devices=CORES)

    x_e = nc.declare_dram_parameter("x", [TOK, H], F32, isOutput=False)
    wq_e = nc.declare_dram_parameter("wq", [H, DPC], F32, isOutput=False)
    wk_e = nc.declare_dram_parameter("wk", [H, HD], F32, isOutput=False)
    wv_e = nc.declare_dram_parameter("wv", [H, HD], F32, isOutput=False)
    wo_e = nc.declare_dram_parameter("wo", [H, H], F32, isOutput=False)
    cos_e = nc.declare_dram_parameter("cosT", [HD, T], F32, isOutput=False)
    sin_e = nc.declare_dram_parameter("sinT", [HD, T], F32, isOutput=False)
    msk_e = nc.declare_dram_parameter("dmask", [128, 4 * 512], F32, isOutput=False)
    id_e = nc.declare_dram_parameter("ident", [128, 128], F32, isOutput=False)
    out_e = nc.declare_dram_parameter("out", [B, OWN, H], F32, isOutput=True)

    with tile.TileContext(nc) as tc:
        with (
            tc.tile_pool(name="dram", bufs=1, space="DRAM") as dpool,
            tc.tile_pool(name="const", bufs=1) as cpool,
        ):
            send = [
                dpool.tile([CORES, DPC, OWN], F32, name=f"send{b}", tag=f"send{b}")
                for b in range(B)
            ]
            recv = [
                dpool.tile([CORES, DPC, OWN], F32, name=f"recv{b}", tag=f"recv{b}")
                for b in range(B)
            ]

            ident = cpool.tile([128, 128], F32)
            nc.sync.dma_start(out=ident[:, :], in_=id_e[:, :])
            cosT = cpool.tile([HD, T], F32)
            nc.sync.dma_start(out=cosT[:, :], in_=cos_e[:, :])
            sinT = cpool.tile([HD, T], F32)
            nc.sync.dma_start(out=sinT[:, :], in_=sin_e[:, :])
            dmask = cpool.tile([128, 4 * 512], F32)
            nc.sync.dma_start(out=dmask[:, :], in_=msk_e[:, :])

            ones_f32 = cpool.tile([128, 1], F32)
            nc.vector.memset(ones_f32[:, :], 1.0)
            ones_col = cpool.tile([128, 1], F32R)
            nc.vector.tensor_copy(ones_col[:, :], ones_f32[:, :])

            # weights -> f32r; emission deferred so the first x tiles win the
            # DMA queue and the PE starts transposing immediately
            wq_r = cpool.tile([128, 16, DPC], F32R)
            wk_r = cpool.tile([128, 16, HD], F32R)
            wv_r = cpool.tile([128, 16, HD], F32R)

            def emit_weight_loads():
                for k in range(16):
                    stg = cpool.tile([128, DPC], F32, tag="wstg", bufs=2, name="wstg")
                    nc.sync.dma_start(
                        out=stg[:, :], in_=wq_e[k * 128 : (k + 1) * 128, :]
                    )
                    nc.vector.tensor_copy(wq_r[:, k, :], stg[:, :])
                    stg2 = cpool.tile([128, HD], F32, tag="wstg2", bufs=2, name="wstg2")
                    nc.sync.dma_start(
                        out=stg2[:, :], in_=wk_e[k * 128 : (k + 1) * 128, :]
                    )
                    nc.gpsimd.tensor_copy(wk_r[:, k, :], stg2[:, :])
                    stg3 = cpool.tile([128, HD], F32, tag="wstg3", bufs=2, name="wstg3")
                    nc.sync.dma_start(
                        out=stg3[:, :], in_=wv_e[k * 128 : (k + 1) * 128, :]
                    )
                    nc.gpsimd.tensor_copy(wv_r[:, k, :], stg3[:, :])

            for b in range(B):
                with tc.tile_pool(name=f"pb{b}", bufs=1) as pp:
                    qTr = pp.tile([128, HPC, T], F32R)
                    kTr = pp.tile([128, T], F32R)
                    vtm = pp.tile([128, 16, HD], F32R)  # token-major v tiles

                    with tc.tile_pool(name=f"pp{b}", bufs=2, space="PSUM") as ppsum:
                        for ch in range(4):  # 512-token chunks of this batch
                            row0 = b * T + ch * 512
                            # xT chunk, layout [tt, ft, 128tok]
                            xT = pp.tile(
                                [128, 4, 16, 128], F32R, tag="xT", bufs=1, name="xT"
                            )
                            for tt in range(4):
                                for fb in range(4):
                                    xin = pp.tile(
                                        [128, 512], F32, tag="xin", bufs=6, name="xin"
                                    )
                                    nc.sync.dma_start(
                                        out=xin[:, :],
                                        in_=x_e[
                                            row0 + tt * 128 : row0 + (tt + 1) * 128,
                                            fb * 512 : (fb + 1) * 512,
                                        ],
                                    )
                                    bank = ppsum.tile(
                                        [128, 512], F32, tag="tr", name="trbank"
                                    )
                                    for j in range(4):
                                        nc.tensor.transpose(
                                            bank[:, j * 128 : (j + 1) * 128],
                                            xin[:, j * 128 : (j + 1) * 128],
                                            ident[:, :],
                                        )
                                    nc.vector.tensor_copy(
                                        xT[:, tt, fb * 4 : (fb + 1) * 4, :], bank[:, :]
                                    )

                            if b == 0 and ch == 0:
                                emit_weight_loads()

                            tsl = slice(ch * 512, (ch + 1) * 512)

                            def rope(dst, ps):
                                sb = pp.tile([128, 512], F32, tag="rsb", bufs=2, name="rsb")
                                nc.vector.tensor_copy(sb[:, :], ps[:, :])
                                rot = pp.tile([128, 512], F32, tag="rrot", bufs=2, name="rrot")
                                nc.sync.dma_start(out=rot[0:64, :], in_=sb[64:128, :])
                                nc.sync.dma_start(out=rot[64:128, :], in_=sb[0:64, :])
                                t1 = pp.tile([128, 512], F32, tag="rt1", bufs=2, name="rt1")
                                nc.vector.tensor_mul(t1[:, :], sb[:, :], cosT[:, tsl])
                                t2 = pp.tile([128, 512], F32, tag="rt2", bufs=2, name="rt2")
                                nc.vector.tensor_mul(t2[:, :], rot[:, :], sinT[:, tsl])
                                nc.vector.tensor_add(dst, t1[:, :], t2[:, :])

                            for m in range(HPC):
                                psq = ppsum.tile([128, 512], F32, tag="pj", name="psq")
                                for k in range(16):
                                    nc.tensor.matmul(
                                        psq[:, :],
                                        wq_r[:, k, m * 128 : (m + 1) * 128],
                                        xT[:, :, k, :],
                                        start=(k == 0),
                                        stop=(k == 15),
                                    )
                                rope(qTr[:, m, tsl], psq)

                            psk = ppsum.tile([128, 512], F32, tag="pj", name="psk")
                            for k in range(16):
                                nc.tensor.matmul(
                                    psk[:, :],
                                    wk_r[:, k, :],
                                    xT[:, :, k, :],
                                    start=(k == 0),
                                    stop=(k == 15),
                                )
                            rope(kTr[:, tsl], psk)

                            psv = ppsum.tile([128, 512], F32, tag="pj", name="psv")
                            for k in range(16):
                                nc.tensor.matmul(
                                    psv[:, :],
                                    wv_r[:, k, :],
                                    xT[:, :, k, :],
                                    start=(k == 0),
                                    stop=(k == 15),
                                )
                            vsb = pp.tile([128, 512], F32, tag="vsb", bufs=2, name="vsb")
                            nc.vector.tensor_copy(vsb[:, :], psv[:, :])
                            vbank = ppsum.tile([128, 512], F32, tag="tr", name="vbank")
                            for s in range(4):
                                nc.tensor.transpose(
                                    vbank[:, s * 128 : (s + 1) * 128],
                                    vsb[:, s * 128 : (s + 1) * 128],
                                    ident[:, :],
                                )
                            nc.vector.tensor_copy(
                                vtm[:, ch * 4 : (ch + 1) * 4, :], vbank[:, :]
                            )

                    # ---- attention for this batch (sw-pipelined, lookahead 2) ----
                    with (
                        tc.tile_pool(name=f"ap{b}", bufs=2, space="PSUM") as apsum,
                        tc.tile_pool(name=f"ab{b}", bufs=1) as ap,
                    ):
                        for h in range(HPC):
                            for qb in range(4):
                                qsl = slice(qb * 512, (qb + 1) * 512)
                                nkt = 4 * (qb + 1)
                                pso = apsum.tile([128, 512], F32, tag="o", name="pso")
                                psl = apsum.tile([1, 512], F32, tag="l", name="psl")
                                pts = {}

                                def score(kt):
                                    pss = apsum.tile(
                                        [128, 512], F32, tag="s", bufs=4, name="pss"
                                    )
                                    nc.tensor.matmul(
                                        pss[:, :],
                                        kTr[:, kt * 128 : (kt + 1) * 128],
                                        qTr[:, h, qsl],
                                        start=True,
                                        stop=True,
                                    )
                                    r = kt - 4 * qb
                                    pT = ap.tile(
                                        [128, 512], F32R, tag="pT", bufs=6, name="pT"
                                    )
                                    if r >= 0:
                                        et = ap.tile(
                                            [128, 512], F32, tag="et", bufs=3, name="et"
                                        )
                                        nc.scalar.activation(et[:, :], pss[:, :], EXP)
                                        nc.vector.tensor_mul(
                                            pT[:, :],
                                            et[:, :],
                                            dmask[:, r * 512 : (r + 1) * 512],
                                        )
                                    else:
                                        nc.scalar.activation(pT[:, :], pss[:, :], EXP)
                                    pts[kt] = pT

                                def pv(kt):
                                    pT = pts.pop(kt)
                                    nc.tensor.matmul(
                                        pso[:, :],
                                        vtm[:, kt, :],
                                        pT[:, :],
                                        start=(kt == 0),
                                        stop=(kt == nkt - 1),
                                    )
                                    nc.tensor.matmul(
                                        psl[0:1, :],
                                        ones_col[:, :],
                                        pT[:, :],
                                        start=(kt == 0),
                                        stop=(kt == nkt - 1),
                                    )

                                LA = 2  # scores run this many k-tiles ahead of PV
                                for kt in range(nkt):
                                    score(kt)
                                    if kt >= LA:
                                        pv(kt - LA)
                                for kt in range(max(0, nkt - LA), nkt):
                                    pv(kt)

                                linv = ap.tile([1, 512], F32, tag="li", bufs=2, name="linv")
                                nc.vector.reciprocal(linv[:, :], psl[0:1, :])
                                lbc = ap.tile([128, 512], F32, tag="lbc", bufs=2, name="lbc")
                                nc.gpsimd.partition_broadcast(lbc[:, :], linv[0:1, :])
                                aout = ap.tile([128, 512], F32, tag="ao", bufs=2, name="aout")
                                nc.vector.tensor_mul(aout[:, :], pso[:, :], lbc[:, :])
                                for half in range(2):
                                    j = 2 * qb + half
                                    nc.sync.dma_start(
                                        out=send[b][j, h * 128 : (h + 1) * 128, :],
                                        in_=aout[:, half * 256 : (half + 1) * 256],
                                    )

                nc.gpsimd.collective_compute(
                    "AllToAll",
                    mybir.AluOpType.bypass,
                    replica_groups=[list(range(CORES))],
                    ins=[send[b].opt()],
                    outs=[recv[b].opt()],
                )

            # ---- o-projection on own tokens ----
            with (
                tc.tile_pool(name="op", bufs=1) as op,
                tc.tile_pool(name="ops", bufs=1, space="PSUM") as opsum,
            ):
                ao_r = op.tile([128, B, 16, OWN], F32R)
                for b in range(B):
                    for k in range(16):
                        stg = op.tile([128, OWN], F32, tag="aostg", bufs=3, name="aostg")
                        nc.sync.dma_start(
                            out=stg[:, :],
                            in_=recv[b][k // 2, (k % 2) * 128 : (k % 2) * 128 + 128, :],
                        )
                        nc.vector.tensor_copy(ao_r[:, b, k, :], stg[:, :])

                for nb in range(4):
                    nsl = slice(nb * 512, (nb + 1) * 512)
                    # all 16 wo k-tiles of this n-block stay resident so the
                    # two batches reuse them (A2A of b=1 hides behind b=0 MMs)
                    wrs = []
                    for k in range(16):
                        wst = op.tile([128, 512], F32, tag="wst", bufs=4, name="wst")
                        nc.sync.dma_start(
                            out=wst[:, :], in_=wo_e[k * 128 : (k + 1) * 128, nsl]
                        )
                        wr = op.tile(
                            [128, 512], F32R, tag=f"wr{k}", bufs=2, name=f"wr{k}"
                        )
                        nc.vector.tensor_copy(wr[:, :], wst[:, :])
                        wrs.append(wr)
                    for b in range(B):
                        for tt in range(2):
                            ps = opsum.tile(
                                [128, 512],
                                F32,
                                tag=f"op{2 * b + tt}",
                                bufs=2,
                                name=f"ops{2 * b + tt}",
                            )
                            for k in range(16):
                                nc.tensor.matmul(
                                    ps[:, :],
                                    ao_r[:, b, k, tt * 128 : (tt + 1) * 128],
                                    wrs[k][:, :],
                                    start=(k == 0),
                                    stop=(k == 15),
                                )
                            osb = op.tile([128, 512], F32, tag="osb", bufs=4, name="osb")
                            nc.vector.tensor_copy(osb[:, :], ps[:, :])
                            nc.sync.dma_start(
                                out=out_e[b, tt * 128 : (tt + 1) * 128, nsl],
                                in_=osb[:, :],
                            )

    nc.compile()
    return nc


def _host_tables():
    inv_freq = 1.0 / (ROPE_THETA ** (np.arange(0, HD, 2, dtype=np.float64) / HD))
    pos = np.arange(T, dtype=np.float64)
    freqs = pos[:, None] * inv_freq[None, :]  # [T, 64]
    emb = np.concatenate([freqs, freqs], axis=-1)  # [T, 128]
    cosT = np.cos(emb).T.astype(np.float32)  # [128, T]
    sinT = np.sin(emb).T.astype(np.float32)
    sinT[:64, :] *= -1.0  # sign of the rotate-half fold
    # diagonal-band causal masks: dmask[r][k', q'] = 1 if q' >= 128 r + k'
    q = np.arange(512)[None, :]
    kk = np.arange(128)[:, None]
    dm = np.concatenate(
        [(q >= 128 * r + kk).astype(np.float32) for r in range(4)], axis=1
    )  # [128, 2048]
    ident = np.eye(128, dtype=np.float32)
    return cosT, sinT, dm, ident


def _run(inputs, trace=False):
    if "nc" not in _CACHE:
        _CACHE["nc"] = _build_nc()
    nc = _CACHE["nc"]

    x = np.ascontiguousarray(inputs["x"], dtype=np.float32).reshape(TOK, H)
    wq = np.asarray(inputs["wq"], dtype=np.float32) * np.float32(SCALE)
    wk = np.asarray(inputs["wk"], dtype=np.float32)
    wv = np.asarray(inputs["wv"], dtype=np.float32)
    wo = np.ascontiguousarray(inputs["wo"], dtype=np.float32)
    cosT, sinT, dm, ident = _host_tables()

    in_maps = []
    for c in range(CORES):
        kv = c // 2
        in_maps.append(
            {
                "x": x,
                "wq": np.ascontiguousarray(wq[:, c * DPC : (c + 1) * DPC]),
                "wk": np.ascontiguousarray(wk[:, kv * HD : (kv + 1) * HD]),
                "wv": np.ascontiguousarray(wv[:, kv * HD : (kv + 1) * HD]),
                "wo": wo,
                "cosT": cosT,
                "sinT": sinT,
                "dmask": dm,
                "ident": ident,
            }
        )

    res = run_bass_kernel_spmd(nc, in_maps, core_ids=list(range(CORES)), trace=trace)
    out = np.empty((B, T, H), dtype=np.float32)
    for c in range(CORES):
        o = res.results[c]["out"]  # [B, OWN, H]
        for b in range(B):
            out[b, c * OWN : (c + 1) * OWN, :] = o[b]
    return out, res


def kernel(**inputs) -> np.ndarray:
    out, _ = _run(inputs, trace=False)
    return out
